# revision 10
# baseline (speedup 1.0000x reference)
"""Trainium2 Bass kernel for nn_MultiHeadAttention_24378234372210.

Exploits the algebraic structure of the reference:
  - attn_out = softmax(scores).sum(axis=-1) over the softmax axis == 1.0 exactly.
  - The context einsum factorizes: context = (sum_k attn) @ (sum_v Vc).
  - get_con_vecs' raw reshapes mean Qc/Kc rows are gathers of only 32 special
    rows of (Q@WQ+bQ)/(K@WK+bK) per batch, gated by 0/1 masks that are
    block-constant over cc=c//8:
      scores[b,h,q,k,c] = gq(h,q,cc)*gk(h,k,cc)*S0[b, q%8%4, k%8%4, c]
    with S0 of shape (4,4,32) per batch.
  - softmax rows fall into a tiny number of (head, gate-pattern) classes; the
    mask-dependent part reduces to per-class matmuls over the key axis.

Sharding: core j handles batch b=j//4 and a 64-row slice of the q' (=aq-major)
reordering for the y_n output; attn_out (all ones) is written 2 (b,h)-slices
per core.  All per-core specialization is carried in input DATA (slot tables),
so one SPMD program serves all 8 cores.
"""
import functools
import numpy as np

B, S, DM, H, DK, CUT = 2, 256, 512, 8, 64, 16
NC2 = 2 * CUT            # 32 context slots
NSK = 4                  # unit slots per head
NSLOT = H * NSK          # 32 total slots
LN_EPS = 1e-5
N_CORES = 8

SROWS = np.concatenate([np.arange(16), np.arange(240, 256)])  # special proj rows
SIDX = {int(s): i for i, s in enumerate(SROWS)}

# index maps between natural and aq-major orderings
#   q' = a*32 + qh  <->  q = qh*8 + a      (same for k'/k)
QP2Q = np.array([(qp % 32) * 8 + qp // 32 for qp in range(256)], np.int64)


def _pos(ap, cc):
    return ap * 64 + 30 + cc


@functools.cache
def _tables():
    """All host-side constant tables (data-independent)."""
    # gates G[h, idx, cc]
    G = np.zeros((H, 256, 4), np.float32)
    for h in range(H):
        for idx in range(256):
            a, i = idx % 8, h * 32 + idx // 8
            for cc in range(4):
                if a <= 3:
                    G[h, idx, cc] = 1.0 if i >= 256 - _pos(a, cc) else 0.0
                else:
                    G[h, idx, cc] = 1.0 if i <= 255 - _pos(a - 4, cc) else 0.0

    # global unit list: (h, aq', pattern) with nonzero pattern; IND over q'
    units, unit_idx, ind = [], {}, []
    for h in range(H):
        for qp in range(256):
            a, qh = qp // 32, qp % 32
            pat = tuple(int(x) for x in G[h, qh * 8 + a])
            if not any(pat):
                continue
            key = (h, a % 4, pat)
            if key not in unit_idx:
                unit_idx[key] = len(units)
                units.append(key)
                ind.append(np.zeros(256, bool))
            ind[unit_idx[key]][qp] = True

    # per-core-slice slot assignment: slot s = h*NSK + k
    # slots[cs][s] = (aqp, pat, ind_vec) or None
    slots = []
    for cs in range(4):
        lo, hi = cs * 64, cs * 64 + 64
        sl = [None] * NSLOT
        for u, (h, aqp, pat) in enumerate(units):
            if not ind[u][lo:hi].any():
                continue
            for k in range(NSK):
                if sl[h * NSK + k] is None:
                    sl[h * NSK + k] = (aqp, pat, ind[u])
                    break
            else:
                raise RuntimeError("slot overflow")
        slots.append(sl)

    # Gk in k' order: Gkp[h, kp, cc]
    Gkp = G[:, QP2Q, :]  # (H, 256, 4)

    # OHall [32, 128]: gather projections rows -> RQ/RK rows.
    # group g in 0..7: RQ rows g*16..g*16+16 = (ap=g//2, cc=(g%2)*2+ccl, cm),
    # source proj row SIDX[(pos%32)*8+cm], rhs col block = g*64.
    OHall = np.zeros((32, 8 * 128), np.float32)
    for g in range(8):
        ap = g // 2
        for ccl in range(2):
            cc = (g % 2) * 2 + ccl
            p = _pos(ap, cc)
            for cm in range(8):
                r = g * 16 + ccl * 8 + cm
                OHall[SIDX[(p % 32) * 8 + cm], g * 128 + r] = 1.0

    # OAK [4, 128]: ak'(local k') one-hot, same for both k'-tiles
    OAK = np.zeros((4, 128), np.float32)
    for p in range(128):
        OAK[(p // 32) % 4, p] = 1.0

    # ncnt[h, ak', cc] = count of v with (v%8)%4==ak' and gate on
    ncnt = np.zeros((H, 4, 4), np.float32)
    for h in range(H):
        for v in range(256):
            ncnt[h, (v % 8) % 4] += G[h, v]

    # M tiles [2][128, 128]: Wsum_all[(hl,c), d] = sum_{(ak',c')} M[(ak',c'),(hl,c)]*RK[(ak',c'),d]
    M = np.zeros((2, 128, 128), np.float32)
    for t in range(2):
        for hl in range(4):
            h = t * 4 + hl
            for akp in range(4):
                for c in range(32):
                    M[t, akp * 32 + c, hl * 32 + c] = ncnt[h, akp, c // 8]

    # per-slice SEL u8 [128, NSLOT*32] and Gkc u8 [2, 128, NSLOT*32]
    SELs, GKCs = [], []
    for cs in range(4):
        sel = np.zeros((128, NSLOT * 32), np.uint8)
        gkc = np.zeros((2, 128, NSLOT * 32), np.uint8)
        for s, ent in enumerate(slots[cs]):
            if ent is None:
                continue
            aqp, pat, _ = ent
            h = s // NSK
            for c in range(32):
                sel[aqp * 32 + c, s * 32 + c] = 1
                if pat[c // 8]:
                    for t in range(2):
                        kp = np.arange(128) + t * 128
                        gkc[t, :, s * 32 + c] = Gkp[h, kp, c // 8].astype(np.uint8)
        SELs.append(sel)
        GKCs.append(gkc)

    return dict(G=G, slots=slots, OHall=OHall, OAK=OAK, M=M, SELs=SELs, GKCs=GKCs)


def _host_inputs(core, Q, K, attn_mask, WQ, WK, Wo, bQ, bK, bo, gamma, beta):
    """Per-core input arrays."""
    t = _tables()
    b, cs = core // 4, core % 4
    qsT = np.ascontiguousarray(Q[b][SROWS].T.reshape(4, 128, 32).transpose(1, 0, 2)
                               .reshape(128, 128))
    ksT = np.ascontiguousarray(K[b][SROWS].T.reshape(4, 128, 32).transpose(1, 0, 2)
                               .reshape(128, 128))
    qrows = QP2Q[cs * 64:(cs + 1) * 64]
    qres = np.ascontiguousarray(Q[b][qrows])
    # nmu[t, p, s*64+i] = (1-mask[b, q_i, k(kp)]) * IND_s[q'_i]
    nm = 1.0 - attn_mask[b].astype(np.float32)        # (q, k)
    nmT = nm[:, QP2Q].T[:, qrows]                      # (k'=256, 64)
    nmu = np.zeros((2, 128, NSLOT * 64), np.uint8)
    for s, ent in enumerate(t["slots"][cs]):
        if ent is None:
            continue
        _, _, ind = ent
        iv = ind[cs * 64:(cs + 1) * 64].astype(np.uint8)  # (64,)
        blk = (nmT * iv[None, :]).astype(np.uint8)        # (256, 64)
        nmu[0, :, s * 64:(s + 1) * 64] = blk[:128]
        nmu[1, :, s * 64:(s + 1) * 64] = blk[128:]
    vecs = np.concatenate([gamma, beta, bo, bQ, bK]).reshape(1, 5 * DM)
    return dict(
        WQ=WQ, WK=WK, Wo=Wo, qsT=qsT, ksT=ksT, qres=qres,
        vecs=vecs.astype(np.float32),
        selu=t["SELs"][cs], gkcu=t["GKCs"][cs], nmu=nmu,
    )


def _build_program():
    import concourse.bacc as bacc
    import concourse.tile as tile
    import concourse.mybir as mybir
    from concourse import bass

    t = _tables()
    f32, u8 = mybir.dt.float32, mybir.dt.uint8
    Alu, Act, Ax = mybir.AluOpType, mybir.ActivationFunctionType, mybir.AxisListType

    nc = bacc.Bacc("TRN2", target_bir_lowering=False, debug=False,
                   enable_asserts=False, num_devices=N_CORES)

    i_WQ = nc.dram_tensor("WQ", [DM, DM], f32, kind="ExternalInput").ap()
    i_WK = nc.dram_tensor("WK", [DM, DM], f32, kind="ExternalInput").ap()
    i_Wo = nc.dram_tensor("Wo", [DM, DM], f32, kind="ExternalInput").ap()
    i_qsT = nc.dram_tensor("qsT", [128, 128], f32, kind="ExternalInput").ap()
    i_ksT = nc.dram_tensor("ksT", [128, 128], f32, kind="ExternalInput").ap()
    i_qres = nc.dram_tensor("qres", [64, DM], f32, kind="ExternalInput").ap()
    i_vecs = nc.dram_tensor("vecs", [1, 5 * DM], f32, kind="ExternalInput").ap()
    i_selu = nc.dram_tensor("selu", [128, NSLOT * 32], u8, kind="ExternalInput").ap()
    i_gkcu = nc.dram_tensor("gkcu", [2, 128, NSLOT * 32], u8, kind="ExternalInput").ap()
    i_nmu = nc.dram_tensor("nmu", [2, 128, NSLOT * 64], u8, kind="ExternalInput").ap()
    o_y = nc.dram_tensor("y_out", [64, DM], f32, kind="ExternalOutput").ap()
    o_ones = nc.dram_tensor("ones_out", [128, 1024], f32, kind="ExternalOutput").ap()

    c_oh = nc.inline_tensor(t["OHall"], name="c_oh")
    c_oak = nc.inline_tensor(t["OAK"], name="c_oak")
    c_m0 = nc.inline_tensor(t["M"][0], name="c_m0")
    c_m1 = nc.inline_tensor(t["M"][1], name="c_m1")

    with tile.TileContext(nc) as tc:
        with (
            tc.tile_pool(name="consts", bufs=1) as cp,
            tc.tile_pool(name="wstream", bufs=3) as wp,
            tc.tile_pool(name="small", bufs=1) as sp,
            tc.tile_pool(name="big", bufs=1) as bp,
        ):
            phase1 = tc.tile_pool(name="ps_p1", bufs=2, space="PSUM")
            ps_proj = phase1.__enter__()
            ps_rr = ps_proj
            # ---- const / input loads ----
            oh_t = cp.tile([32, 8 * 128], f32)
            nc.sync.dma_start(oh_t[:], c_oh[:, :])
            oak_t = cp.tile([4, 128], f32)
            nc.sync.dma_start(oak_t[:], c_oak[:, :])
            m0_t = cp.tile([128, 128], f32)
            nc.sync.dma_start(m0_t[:], c_m0[:, :])
            m1_t = cp.tile([128, 128], f32)
            nc.sync.dma_start(m1_t[:], c_m1[:, :])
            qsT_t = cp.tile([128, 128], f32)
            nc.sync.dma_start(qsT_t[:], i_qsT)
            ksT_t = cp.tile([128, 128], f32)
            nc.sync.dma_start(ksT_t[:], i_ksT)
            vecs_t = cp.tile([1, 5 * DM], f32)
            nc.sync.dma_start(vecs_t[:], i_vecs)

            # ---- attn_out ones (independent) ----
            ones_t = bp.tile([128, 1024], f32)
            nc.gpsimd.memset(ones_t[:], 1.0)
            nc.sync.dma_start(o_ones, ones_t[:])

            # ---- projections: proj = (special rows) @ W + bias ----
            projs = {}
            for nm_, wdram, lhsT, bslice in (
                ("q", i_WQ, qsT_t, 3), ("k", i_WK, ksT_t, 4),
            ):
                pps = ps_proj.tile([32, DM], f32, name=f"pps{nm_}", tag="pps")
                for ci in range(4):
                    wt = wp.tile([128, DM], f32, name="wt", tag="wt")
                    nc.sync.dma_start(wt[:], wdram[ci * 128:(ci + 1) * 128, :])
                    nc.tensor.matmul(pps[:], lhsT[:, ci * 32:(ci + 1) * 32], wt[:],
                                     start=(ci == 0), stop=(ci == 3))
                bb = sp.tile([32, DM], f32, name=f"bb{nm_}")
                nc.gpsimd.partition_broadcast(
                    bb[:], vecs_t[:, bslice * DM:(bslice + 1) * DM])
                proj = sp.tile([32, DM], f32, name=f"proj{nm_}")
                nc.vector.tensor_tensor(proj[:], pps[:], bb[:], Alu.add)
                projs[nm_] = proj

            # ---- RQ / RK gathers: PSUM-accumulated full-width one-hots ----
            rr = {}
            for nm_ in ("q", "k"):
                rps = ps_rr.tile([128, 64], f32, name="rps", tag="rps")
                for g in range(8):
                    nc.tensor.matmul(rps[:],
                                     oh_t[:, g * 128:(g + 1) * 128],
                                     projs[nm_][:, g * 64:(g + 1) * 64],
                                     start=(g == 0), stop=(g == 7))
                r_sb = sp.tile([128, 64], f32, name=f"r{nm_}")
                nc.scalar.copy(r_sb[:], rps[:])
                rr[nm_] = r_sb
            RQ, RK = rr["q"], rr["k"]

            # ---- S0 and E0m1 = exp(S0/8) - 1 ----
            s0 = sp.tile([128, 4], f32)
            rk4 = sp.tile([128, 64], f32)
            prod = sp.tile([128, 64], f32)
            for akp in range(4):
                for aqp in range(4):
                    nc.vector.tensor_copy(rk4[aqp * 32:(aqp + 1) * 32, :],
                                          RK[akp * 32:(akp + 1) * 32, :])
                nc.vector.scalar_tensor_tensor(
                    prod[:], RQ[:], 1.0, rk4[:], Alu.bypass, Alu.mult,
                    accum_out=s0[:, akp:akp + 1])
            e0m1 = sp.tile([128, 4], f32)
            nc.scalar.activation(e0m1[:], s0[:], Act.Exp, scale=0.125)
            nc.vector.tensor_scalar_add(e0m1[:], e0m1[:], -1.0)

            # ---- Wsum (split to per-h [32,64] tiles so ctx lhsT is base-0) ----
            ws_h = [sp.tile([32, 64], f32, name=f"wsh{h}") for h in range(H)]
            for tt in range(2):
                wps = ps_rr.tile([128, 64], f32, name="wps", tag="wps")
                nc.tensor.matmul(wps[:], (m0_t if tt == 0 else m1_t)[:], RK[:],
                                 start=True, stop=True)
                for hl in range(4):
                    nc.scalar.copy(ws_h[tt * 4 + hl][:],
                                   wps[hl * 32:(hl + 1) * 32, :])

            phase1.__exit__(None, None, None)
            phase2 = tc.tile_pool(name="ps_p2", bufs=2, space="PSUM")
            ps_e = phase2.__enter__()

            # ---- SEL/Gkc converts ----
            selu_t = sp.tile([128, NSLOT * 32], u8)
            nc.sync.dma_start(selu_t[:], i_selu)
            sel_f = bp.tile([128, NSLOT * 32], f32)
            nc.gpsimd.tensor_copy(sel_f[:], selu_t[:])

            # ---- E0TU = E0m1^T gathered per slot: [4, NSLOT*32] ----
            e0tu_sb = sp.tile([4, NSLOT * 32], f32)
            for half in range(2):
                cw = NSLOT * 16  # 512
                eps_ = ps_e.tile([4, cw], f32, name="eps", tag="eps")
                nc.tensor.matmul(eps_[:], e0m1[:],
                                 sel_f[:, half * cw:(half + 1) * cw],
                                 start=True, stop=True)
                nc.scalar.copy(e0tu_sb[:, half * cw:(half + 1) * cw], eps_[:])

            # ---- E0M1SEL [128, NSLOT*32] (shared across k'-tiles) ----
            emsel_ps = []
            for half in range(2):
                cw = NSLOT * 16
                mps = ps_e.tile([128, cw], f32, name="mps", tag="mps")
                nc.tensor.matmul(mps[:], oak_t[:],
                                 e0tu_sb[:, half * cw:(half + 1) * cw],
                                 start=True, stop=True)
                emsel_ps.append(mps)

            # ---- per k'-tile: ENUMm1, Z, Rz, DqAll ----
            dqall = []
            for kt in range(2):
                gkcu_t = sp.tile([128, NSLOT * 32], u8, name=f"gkcu{kt}")
                nc.sync.dma_start(gkcu_t[:], i_gkcu[kt, :, :])
                gkc_f = bp.tile([128, NSLOT * 32], f32, name=f"gkcf{kt}")
                nc.gpsimd.tensor_copy(gkc_f[:], gkcu_t[:])
                enumm1 = bp.tile([128, NSLOT * 32], f32, name=f"enumm1{kt}")
                for half in range(2):
                    cw = NSLOT * 16
                    nc.vector.tensor_tensor(
                        enumm1[:, half * cw:(half + 1) * cw], emsel_ps[half][:],
                        gkc_f[:, half * cw:(half + 1) * cw], Alu.mult)
                zm = sp.tile([128, NSLOT], f32, name=f"zm{kt}")
                nc.vector.reduce_sum(
                    zm[:], enumm1[:].rearrange("p (u c) -> p u c", c=32), axis=Ax.X)
                nc.vector.tensor_scalar_add(zm[:], zm[:], 32.0)
                rz = sp.tile([128, NSLOT], f32, name=f"rz{kt}")
                nc.vector.reciprocal(rz[:], zm[:])
                dq = bp.tile([128, NSLOT * 32], f32, name=f"dq{kt}")
                rzb = rz[:].unsqueeze(2).broadcast_to([128, NSLOT, 32])
                nc.vector.scalar_tensor_tensor(
                    dq[:].rearrange("p (u c) -> p u c", c=32),
                    enumm1[:].rearrange("p (u c) -> p u c", c=32),
                    1.0, rzb, Alu.add, Alu.mult)
                nc.scalar.activation(dq[:], dq[:], Act.Copy, bias=-1.0 / 32.0)
                dqall.append(dq)

            phase2.__exit__(None, None, None)
            phase3 = tc.tile_pool(name="ps_p3", bufs=2, space="PSUM")
            ps_at = phase3.__enter__()
            ps_ctx = ps_at
            phase3y = tc.tile_pool(name="ps_p3y", bufs=1, space="PSUM")
            ps_y = phase3y.__enter__()

            # ---- NMU convert ----
            nmf = []
            for kt in range(2):
                nmu_t = sp.tile([128, NSLOT * 64], u8, name=f"nmu{kt}")
                nc.sync.dma_start(nmu_t[:], i_nmu[kt, :, :])
                f = bp.tile([128, NSLOT * 64], f32, name=f"nmf{kt}")
                eng = nc.gpsimd if kt == 0 else nc.vector
                eng.tensor_copy(f[:], nmu_t[:])
                nmf.append(f)

            # ---- CORRT matmuls + A^T assembly ----
            at_sb = sp.tile([32, H * 64], f32)
            for h in range(H):
                atp = ps_at.tile([32, 64], f32, name="atp", tag="atp")
                for k in range(NSK):
                    s = h * NSK + k
                    for kt in range(2):
                        nc.tensor.matmul(
                            atp[:], dqall[kt][:, s * 32:(s + 1) * 32],
                            nmf[kt][:, s * 64:(s + 1) * 64],
                            start=(k == 0 and kt == 0),
                            stop=(k == NSK - 1 and kt == 1))
                nc.scalar.activation(at_sb[:, h * 64:(h + 1) * 64], atp[:],
                                     Act.Copy, bias=8.0)

            # ---- ctxT ----
            ctxT = [sp.tile([128, 64], f32, name=f"ctxT{i}") for i in range(4)]
            for h in range(H):
                cps = ps_ctx.tile([64, 64], f32, name="cps", tag="cps")
                nc.tensor.matmul(cps[:], ws_h[h][:],
                                 at_sb[:, h * 64:(h + 1) * 64], start=True, stop=True)
                nc.scalar.copy(ctxT[h // 2][(h % 2) * 64:(h % 2 + 1) * 64, :], cps[:])

            # ---- Wo matmul + residual + LayerNorm ----
            yps = ps_y.tile([64, DM], f32)
            for ci in range(4):
                wt = wp.tile([128, DM], f32, name="wt2", tag="wt")
                nc.sync.dma_start(wt[:], i_Wo[ci * 128:(ci + 1) * 128, :])
                nc.tensor.matmul(yps[:], ctxT[ci][:, :], wt[:],
                                 start=(ci == 0), stop=(ci == 3))
            qres_t = sp.tile([64, DM], f32)
            nc.sync.dma_start(qres_t[:], i_qres)
            bob = sp.tile([64, DM], f32)
            nc.gpsimd.partition_broadcast(bob[:], vecs_t[:, 2 * DM:3 * DM])
            y1 = sp.tile([64, DM], f32)
            nc.vector.tensor_tensor(y1[:], yps[:], qres_t[:], Alu.add)
            ysum = sp.tile([64, 1], f32)
            y2 = sp.tile([64, DM], f32)
            nc.vector.scalar_tensor_tensor(y2[:], y1[:], 1.0, bob[:],
                                           Alu.bypass, Alu.add, accum_out=ysum[:])
            mu = sp.tile([64, 1], f32)
            nc.scalar.mul(mu[:], ysum[:], 1.0 / DM)
            cent = sp.tile([64, DM], f32)
            nc.vector.tensor_scalar(cent[:], y2[:], mu[:], None, Alu.subtract)
            sq = sp.tile([64, DM], f32)
            varsum = sp.tile([64, 1], f32)
            nc.vector.scalar_tensor_tensor(sq[:], cent[:], 1.0, cent[:],
                                           Alu.bypass, Alu.mult,
                                           accum_out=varsum[:])
            var_eps = sp.tile([64, 1], f32)
            nc.scalar.activation(var_eps[:], varsum[:], Act.Copy, scale=1.0 / DM,
                                 bias=LN_EPS)
            sd = sp.tile([64, 1], f32)
            nc.scalar.activation(sd[:], var_eps[:], Act.Sqrt)
            rstd = sp.tile([64, 1], f32)
            nc.vector.reciprocal(rstd[:], sd[:])
            gb = sp.tile([64, DM], f32)
            nc.gpsimd.partition_broadcast(gb[:], vecs_t[:, 0:DM])
            bb2 = sp.tile([64, DM], f32)
            nc.gpsimd.partition_broadcast(bb2[:], vecs_t[:, DM:2 * DM])
            t3 = sp.tile([64, DM], f32)
            nc.vector.tensor_scalar(t3[:], cent[:], rstd[:], None, Alu.mult)
            t4 = sp.tile([64, DM], f32)
            nc.vector.tensor_tensor(t4[:], t3[:], gb[:], Alu.mult)
            yn = sp.tile([64, DM], f32)
            nc.vector.tensor_tensor(yn[:], t4[:], bb2[:], Alu.add)
            nc.sync.dma_start(o_y, yn[:])

            phase3y.__exit__(None, None, None)
            phase3.__exit__(None, None, None)

    nc.compile()
    return nc


_PROGRAM = None


def _get_program():
    global _PROGRAM
    if _PROGRAM is None:
        _PROGRAM = _build_program()
    return _PROGRAM


def kernel(Q, K, V, attn_mask, WQ, bQ, WK, bK, WV, bV, Wo, bo, gamma, beta,
           **_unused):
    from concourse.bass_utils import run_bass_kernel_spmd

    Q = np.asarray(Q, np.float32)
    K = np.asarray(K, np.float32)
    attn_mask = np.asarray(attn_mask)
    args = dict(WQ=np.asarray(WQ, np.float32), WK=np.asarray(WK, np.float32),
                Wo=np.asarray(Wo, np.float32), bQ=np.asarray(bQ, np.float32),
                bK=np.asarray(bK, np.float32), bo=np.asarray(bo, np.float32),
                gamma=np.asarray(gamma, np.float32),
                beta=np.asarray(beta, np.float32))
    in_maps = [_host_inputs(j, Q, K, attn_mask, **args) for j in range(N_CORES)]

    nc = _get_program()
    res = run_bass_kernel_spmd(nc, in_maps, core_ids=list(range(N_CORES)))

    y_n = np.zeros((B, S, DM), np.float32)
    attn_out = np.zeros((B, H, S, S), np.float32)
    ao_flat = attn_out.reshape(B * H, S * S)
    for j in range(N_CORES):
        b, cs = j // 4, j % 4
        y_n[b, QP2Q[cs * 64:(cs + 1) * 64], :] = res.results[j]["y_out"]
        ao_flat[2 * j:2 * j + 2] = res.results[j]["ones_out"].reshape(2, S * S)
    return y_n, attn_out
